# revision 10
# baseline (speedup 1.0000x reference)
"""Trainium2 Bass kernel for nn_ComplexNet: out = x @ M_r.T

Reference math: x_imag = 0, so only M_r (the real coefficient matrix,
[2, 10], built from psi/A via a tiny einsum) matters:
    out[t, k] = sum_a x[t, a] * M_r[k, a]

Strategy (memory-bound, ~24.6 MB HBM traffic per core):
  - Data-parallel over 8 NeuronCores: pad T 4,000,000 -> 4,096,000, each core
    takes a contiguous 512,000-row shard.
  - Host computes M_r (tiny einsum over psi/A) in float64, ships it as a
    [128, 20] replicated input plus a [128, 128] identity.
  - Per core, 8 tiles of [128 partitions x 5000] in natural layout
    (partition p owns 500 consecutive rows -> 20 KB contiguous per
    partition, full-rate 2.56 MB DMAs).
  - Gather pass (DVE/ACT split): 10 strided copies per tile rearrange
    (m, a)-interleaved -> a-major contiguous, rounding to float32r.
    (Strided moving operands run the PE at 2.5 cyc/row; contiguous at 1.1.)
  - TensorEngine: per (k, a) one matmul, stationary M[k,a]*I_128 (float32r,
    self-loading LDWEIGHTS overlaps the moving stream), moving contiguous
    [128, 500], accumulating the a-sum in PSUM.
  - PSUM -> SBUF copies interleave the two k columns; one 512 KB DMA out
    per tile.

kernel(**inputs) takes the FULL unsharded inputs, returns the FULL
[4_000_000, 2] float32 output.
"""

import sys

import numpy as np

if "/opt/trn_rl_repo" not in sys.path:
    sys.path.insert(0, "/opt/trn_rl_repo")

from contextlib import ExitStack

import concourse.bacc as bacc
import concourse.tile as tile
from concourse import mybir
from concourse.bass_utils import run_bass_kernel_spmd

T = 4_000_000
N_FEAT = 10
N_CORES = 8
P = 128

N_PER_PART = 500           # rows per partition per tile = moving free dim
N_TILES = 8                # tiles per core
R = P * N_PER_PART * N_TILES   # 512_000 rows per core
T_PAD = R * N_CORES            # 4_096_000

DT = mybir.dt.float32
DT_R = mybir.dt.float32r

_CACHE = {}


def _build():
    if "nc" in _CACHE:
        return _CACHE["nc"]
    nc = bacc.Bacc("TRN2", target_bir_lowering=False, debug=False,
                   num_devices=N_CORES)
    x_d = nc.dram_tensor("x", [R, N_FEAT], DT, kind="ExternalInput")
    m_d = nc.dram_tensor("m", [P, 20], DT, kind="ExternalInput")
    id_d = nc.dram_tensor("idm", [P, P], DT, kind="ExternalInput")
    o_d = nc.dram_tensor("out", [R, 2], DT, kind="ExternalOutput")

    x_ap = x_d.ap().rearrange("(i p m) a -> i p (m a)", i=N_TILES, p=P)
    o_ap = o_d.ap().rearrange("(i p m) k -> i p (m k)", i=N_TILES, p=P)

    NM = N_PER_PART

    with tile.TileContext(nc) as tc, ExitStack() as ctx:
        consts = ctx.enter_context(tc.tile_pool(name="consts", bufs=1))
        xpool = ctx.enter_context(tc.tile_pool(name="xp", bufs=3))
        gpool = ctx.enter_context(tc.tile_pool(name="gp", bufs=3))
        opool = ctx.enter_context(tc.tile_pool(name="op", bufs=3))
        wpool = ctx.enter_context(tc.tile_pool(name="wp", bufs=1))
        psum = ctx.enter_context(tc.tile_pool(name="ps", bufs=3, space="PSUM"))

        # consts via the SWDGE queue so the first x tile owns the Sync ring
        id_sb = consts.tile([P, P], DT)
        nc.gpsimd.dma_start(id_sb[:], id_d.ap())
        m_sb = consts.tile([P, 20], DT)
        nc.gpsimd.dma_start(m_sb[:], m_d.ap())

        # 20 scaled identities W[k*10+a] = M[k,a] * I, rounded to fp32r.
        # On ACT (idle at startup; DVE would delay the first gathers).
        w_sb = wpool.tile([P, 20 * P], DT_R)
        for j in range(20):
            nc.scalar.mul(
                w_sb[:, j * P:(j + 1) * P], id_sb[:], m_sb[:, j:j + 1]
            )

        for i in range(N_TILES):
            x_sb = xpool.tile([P, NM * N_FEAT], DT)
            nc.sync.dma_start(x_sb[:], x_ap[i])
            x3 = x_sb[:].rearrange("p (m a) -> p m a", a=N_FEAT)

            # gather: (m, a) interleaved -> a-major contiguous, cast fp32r.
            # Pair copies (two a-slices per op): src inner dim is an 8-byte
            # contiguous pair, dst writes the two a-major runs.
            xg = gpool.tile([P, NM * N_FEAT], DT_R)
            xg3 = xg[:].rearrange("p (a m) -> p m a", m=NM)
            for a0 in range(0, N_FEAT, 2):
                dst = xg3[:, :, a0:a0 + 2]
                src = x3[:, :, a0:a0 + 2]
                if a0 < 8:
                    nc.vector.tensor_copy(dst, src)
                else:
                    nc.scalar.copy(dst, src)

            o_sb = opool.tile([P, NM * 2], DT)
            o3 = o_sb[:].rearrange("p (m k) -> p m k", k=2)
            for k in range(2):
                ps = psum.tile([P, NM], mybir.dt.float32,
                               name=f"ps_{i}_{k}", tag=f"ps{k}")
                for a in range(N_FEAT):
                    j = k * 10 + a
                    nc.tensor.matmul(
                        ps[:],
                        w_sb[:, j * P:(j + 1) * P],
                        xg[:, a * NM:(a + 1) * NM],
                        start=(a == 0), stop=(a == N_FEAT - 1),
                    )
                nc.scalar.copy(o3[:, :, k], ps[:])

            # Store on ACT's HWDGE ring (qActDynamicHW): full DMA rate and
            # no head-of-line blocking of x prefetches on the Sync ring.
            nc.scalar.dma_start(o_ap[i], o_sb[:])

    nc.compile()
    _CACHE["nc"] = nc
    return nc


def _host_m(psi_real, psi_imag, A_real, A_imag):
    """M_r in float64: the coefficient matrix multiplying x_real."""
    pr = psi_real.astype(np.float64)
    pi = psi_imag.astype(np.float64)
    Ar = A_real.astype(np.float64)
    Ai = A_imag.astype(np.float64)

    def mat(p1, A, p2):
        return np.einsum("i,kija,j->ka", p1, A, p2)

    M = (mat(pr, Ar, pr) - mat(pi, Ai, pr)
         - mat(pr, Ar, pi) + mat(pi, Ai, pi))
    return M.astype(np.float32)   # [2, 10]


def kernel(x, psi_real, psi_imag, A_real, A_imag, _trace=False):
    M = _host_m(psi_real, psi_imag, A_real, A_imag)

    x = np.ascontiguousarray(x, dtype=np.float32)
    x_pad = np.zeros((T_PAD, N_FEAT), dtype=np.float32)
    x_pad[:T] = x

    m_rep = np.tile(M.reshape(1, 20), (P, 1)).astype(np.float32)
    idm = np.eye(P, dtype=np.float32)

    nc = _build()
    in_maps = [
        {"x": x_pad[c * R:(c + 1) * R], "m": m_rep, "idm": idm}
        for c in range(N_CORES)
    ]
    res = run_bass_kernel_spmd(nc, in_maps, core_ids=list(range(N_CORES)),
                               trace=_trace)
    out = np.concatenate([res.results[c]["out"] for c in range(N_CORES)], axis=0)
    if _trace:
        kernel.last_results = res
    return out[:T]


# revision 15
# speedup vs baseline: 1.1053x; 1.1053x over previous
"""Trainium2 Bass kernel for nn_ComplexNet: out = x @ M_r.T

Reference math: x_imag = 0, so only M_r (the real coefficient matrix,
[2, 10], built from psi/A via a tiny einsum) matters:
    out[t, k] = sum_a x[t, a] * M_r[k, a]

Strategy (memory-bound, ~24.6 MB HBM traffic per core):
  - Data-parallel over 8 NeuronCores: pad T 4,000,000 -> 4,096,000, each core
    takes a contiguous 512,000-row shard.
  - Host computes M_r (tiny einsum over psi/A) in float64, ships it as a
    [128, 20] replicated input plus a [128, 128] identity.
  - Per core, 8 tiles of [128 partitions x 5000] in natural layout
    (partition p owns 500 consecutive rows -> 20 KB contiguous per
    partition, full-rate 2.56 MB DMAs).
  - Gather pass (DVE/ACT split): 10 strided copies per tile rearrange
    (m, a)-interleaved -> a-major contiguous, rounding to float32r.
    (Strided moving operands run the PE at 2.5 cyc/row; contiguous at 1.1.)
  - TensorEngine: per (k, a) one matmul, stationary M[k,a]*I_128 (float32r,
    self-loading LDWEIGHTS overlaps the moving stream), moving contiguous
    [128, 500], accumulating the a-sum in PSUM.
  - PSUM -> SBUF copies interleave the two k columns; one 512 KB DMA out
    per tile.

kernel(**inputs) takes the FULL unsharded inputs, returns the FULL
[4_000_000, 2] float32 output.
"""

import sys

import numpy as np

if "/opt/trn_rl_repo" not in sys.path:
    sys.path.insert(0, "/opt/trn_rl_repo")

from contextlib import ExitStack

import concourse.bacc as bacc
import concourse.tile as tile
from concourse import mybir
from concourse.bass_utils import run_bass_kernel_spmd

T = 4_000_000
N_FEAT = 10
N_CORES = 8
P = 128

# rows per partition per tile = matmul moving free dim.  512 fills one
# PSUM bank exactly; the smaller final tile shortens the kernel tail
# (the last tile's compute + store cannot overlap any input transfer).
TILE_NM = [512] * 7 + [416]
R = P * sum(TILE_NM)           # 512_000 rows per core
T_PAD = R * N_CORES            # 4_096_000

DT = mybir.dt.float32
DT_R = mybir.dt.float32r

_CACHE = {}


def _build():
    if "nc" in _CACHE:
        return _CACHE["nc"]
    nc = bacc.Bacc("TRN2", target_bir_lowering=False, debug=False,
                   num_devices=N_CORES)
    x_d = nc.dram_tensor("x", [R, N_FEAT], DT, kind="ExternalInput")
    m_d = nc.dram_tensor("m", [P, 20], DT, kind="ExternalInput")
    id_d = nc.dram_tensor("idm", [P, P], DT, kind="ExternalInput")
    o_d = nc.dram_tensor("out", [R, 2], DT, kind="ExternalOutput")

    x_flat = x_d.ap()
    o_flat = o_d.ap()

    with tile.TileContext(nc) as tc, ExitStack() as ctx:
        consts = ctx.enter_context(tc.tile_pool(name="consts", bufs=1))
        xpool = ctx.enter_context(tc.tile_pool(name="xp", bufs=3))
        gpool = ctx.enter_context(tc.tile_pool(name="gp", bufs=3))
        opool = ctx.enter_context(tc.tile_pool(name="op", bufs=3))
        wpool = ctx.enter_context(tc.tile_pool(name="wp", bufs=1))
        psum = ctx.enter_context(tc.tile_pool(name="ps", bufs=3, space="PSUM"))

        # consts via the SWDGE queue so the first x tile owns the Sync ring
        id_sb = consts.tile([P, P], DT)
        nc.gpsimd.dma_start(id_sb[:], id_d.ap())
        m_sb = consts.tile([P, 20], DT)
        nc.gpsimd.dma_start(m_sb[:], m_d.ap())

        # 20 scaled identities W[k*10+a] = M[k,a] * I, rounded to fp32r.
        # On ACT (idle at startup; DVE would delay the first gathers).
        w_sb = wpool.tile([P, 20 * P], DT_R)
        for j in range(20):
            nc.scalar.mul(
                w_sb[:, j * P:(j + 1) * P], id_sb[:], m_sb[:, j:j + 1]
            )

        base = 0
        for i, NM in enumerate(TILE_NM):
            rows = P * NM
            x_t = x_flat[base:base + rows].rearrange("(p m) a -> p (m a)", p=P)
            o_t = o_flat[base:base + rows].rearrange("(p m) k -> p (m k)", p=P)
            base += rows

            x_sb = xpool.tile([P, NM * N_FEAT], DT)
            nc.sync.dma_start(x_sb[:], x_t)
            x3 = x_sb[:].rearrange("p (m a) -> p m a", a=N_FEAT)

            # gather: (m, a) interleaved -> a-major contiguous, cast fp32r.
            # Pair copies (two a-slices per op): src inner dim is an 8-byte
            # contiguous pair, dst writes the two a-major runs.
            xg = gpool.tile([P, NM * N_FEAT], DT_R)
            xg3 = xg[:].rearrange("p (a m) -> p m a", m=NM)
            for a0 in range(0, N_FEAT, 2):
                dst = xg3[:, :, a0:a0 + 2]
                src = x3[:, :, a0:a0 + 2]
                if a0 < 8:
                    nc.vector.tensor_copy(dst, src)
                else:
                    nc.scalar.copy(dst, src)

            o_sb = opool.tile([P, NM * 2], DT)
            o3 = o_sb[:].rearrange("p (m k) -> p m k", k=2)
            for k in range(2):
                ps = psum.tile([P, NM], mybir.dt.float32,
                               name=f"ps_{i}_{k}", tag=f"ps{k}")
                for a in range(N_FEAT):
                    j = k * 10 + a
                    nc.tensor.matmul(
                        ps[:],
                        w_sb[:, j * P:(j + 1) * P],
                        xg[:, a * NM:(a + 1) * NM],
                        start=(a == 0), stop=(a == N_FEAT - 1),
                    )
                nc.scalar.copy(o3[:, :, k], ps[:])

            # SWDGE (gpsimd) for the store: keeps the Sync queue free to
            # prefetch x tiles, and the gpsimd sequencer is otherwise idle
            # (issuing stores from ACT's HWDGE ring serializes behind its
            # ACTIVATE ops and measures ~7us slower end-to-end).
            nc.gpsimd.dma_start(o_t, o_sb[:])

    nc.compile()
    _CACHE["nc"] = nc
    return nc


def _host_m(psi_real, psi_imag, A_real, A_imag):
    """M_r in float64: the coefficient matrix multiplying x_real."""
    pr = psi_real.astype(np.float64)
    pi = psi_imag.astype(np.float64)
    Ar = A_real.astype(np.float64)
    Ai = A_imag.astype(np.float64)

    def mat(p1, A, p2):
        return np.einsum("i,kija,j->ka", p1, A, p2)

    M = (mat(pr, Ar, pr) - mat(pi, Ai, pr)
         - mat(pr, Ar, pi) + mat(pi, Ai, pi))
    return M.astype(np.float32)   # [2, 10]


def kernel(x, psi_real, psi_imag, A_real, A_imag, _trace=False):
    M = _host_m(psi_real, psi_imag, A_real, A_imag)

    x = np.ascontiguousarray(x, dtype=np.float32)
    x_pad = np.zeros((T_PAD, N_FEAT), dtype=np.float32)
    x_pad[:T] = x

    m_rep = np.tile(M.reshape(1, 20), (P, 1)).astype(np.float32)
    idm = np.eye(P, dtype=np.float32)

    nc = _build()
    in_maps = [
        {"x": x_pad[c * R:(c + 1) * R], "m": m_rep, "idm": idm}
        for c in range(N_CORES)
    ]
    res = run_bass_kernel_spmd(nc, in_maps, core_ids=list(range(N_CORES)),
                               trace=_trace)
    out = np.concatenate([res.results[c]["out"] for c in range(N_CORES)], axis=0)
    if _trace:
        kernel.last_results = res
    return out[:T]


# revision 16
# speedup vs baseline: 1.1552x; 1.0451x over previous
"""Trainium2 Bass kernel for nn_ComplexNet: out = x @ M_r.T

Reference math: x_imag = 0, so only M_r (the real coefficient matrix,
[2, 10], built from psi/A via a tiny einsum) matters:
    out[t, k] = sum_a x[t, a] * M_r[k, a]

Strategy (memory-bound, ~24.6 MB HBM traffic per core):
  - Data-parallel over 8 NeuronCores: pad T 4,000,000 -> 4,096,000, each core
    takes a contiguous 512,000-row shard.
  - Host computes M_r (tiny einsum over psi/A) in float64, ships it as a
    [128, 20] replicated input plus a [128, 128] identity.
  - Per core, 8 tiles of [128 partitions x 5000] in natural layout
    (partition p owns 500 consecutive rows -> 20 KB contiguous per
    partition, full-rate 2.56 MB DMAs).
  - Gather pass (DVE/ACT split): 10 strided copies per tile rearrange
    (m, a)-interleaved -> a-major contiguous, rounding to float32r.
    (Strided moving operands run the PE at 2.5 cyc/row; contiguous at 1.1.)
  - TensorEngine: per (k, a) one matmul, stationary M[k,a]*I_128 (float32r,
    self-loading LDWEIGHTS overlaps the moving stream), moving contiguous
    [128, 500], accumulating the a-sum in PSUM.
  - PSUM -> SBUF copies interleave the two k columns; one 512 KB DMA out
    per tile.

kernel(**inputs) takes the FULL unsharded inputs, returns the FULL
[4_000_000, 2] float32 output.
"""

import sys

import numpy as np

if "/opt/trn_rl_repo" not in sys.path:
    sys.path.insert(0, "/opt/trn_rl_repo")

from contextlib import ExitStack

import concourse.bacc as bacc
import concourse.tile as tile
from concourse import mybir
from concourse.bass_utils import run_bass_kernel_spmd

T = 4_000_000
N_FEAT = 10
N_CORES = 8
P = 128

# rows per partition per tile = matmul moving free dim.  512 fills one
# PSUM bank exactly; all sizes stay >= 256 (float32r full-rate threshold).
# Small FIRST tile: compute starts ~4.5us earlier (shorter first DMA).
# Small LAST tile: shorter tail (its compute + store cannot overlap any
# input transfer).
TILE_NM = [256] + [512] * 6 + [416, 256]
R = P * sum(TILE_NM)           # 512_000 rows per core
T_PAD = R * N_CORES            # 4_096_000

DT = mybir.dt.float32
DT_R = mybir.dt.float32r

_CACHE = {}


def _build():
    if "nc" in _CACHE:
        return _CACHE["nc"]
    nc = bacc.Bacc("TRN2", target_bir_lowering=False, debug=False,
                   num_devices=N_CORES)
    x_d = nc.dram_tensor("x", [R, N_FEAT], DT, kind="ExternalInput")
    m_d = nc.dram_tensor("m", [P, 20], DT, kind="ExternalInput")
    id_d = nc.dram_tensor("idm", [P, P], DT, kind="ExternalInput")
    o_d = nc.dram_tensor("out", [R, 2], DT, kind="ExternalOutput")

    x_flat = x_d.ap()
    o_flat = o_d.ap()

    with tile.TileContext(nc) as tc, ExitStack() as ctx:
        consts = ctx.enter_context(tc.tile_pool(name="consts", bufs=1))
        xpool = ctx.enter_context(tc.tile_pool(name="xp", bufs=3))
        gpool = ctx.enter_context(tc.tile_pool(name="gp", bufs=3))
        opool = ctx.enter_context(tc.tile_pool(name="op", bufs=3))
        wpool = ctx.enter_context(tc.tile_pool(name="wp", bufs=1))
        psum = ctx.enter_context(tc.tile_pool(name="ps", bufs=3, space="PSUM"))

        # consts via the SWDGE queue so the first x tile owns the Sync ring
        id_sb = consts.tile([P, P], DT)
        nc.gpsimd.dma_start(id_sb[:], id_d.ap())
        m_sb = consts.tile([P, 20], DT)
        nc.gpsimd.dma_start(m_sb[:], m_d.ap())

        # 20 scaled identities W[k*10+a] = M[k,a] * I, rounded to fp32r.
        # On ACT (idle at startup; DVE would delay the first gathers).
        w_sb = wpool.tile([P, 20 * P], DT_R)
        for j in range(20):
            nc.scalar.mul(
                w_sb[:, j * P:(j + 1) * P], id_sb[:], m_sb[:, j:j + 1]
            )

        base = 0
        for i, NM in enumerate(TILE_NM):
            rows = P * NM
            x_t = x_flat[base:base + rows].rearrange("(p m) a -> p (m a)", p=P)
            o_t = o_flat[base:base + rows].rearrange("(p m) k -> p (m k)", p=P)
            base += rows

            x_sb = xpool.tile([P, NM * N_FEAT], DT)
            nc.sync.dma_start(x_sb[:], x_t)
            x3 = x_sb[:].rearrange("p (m a) -> p m a", a=N_FEAT)

            # gather: (m, a) interleaved -> a-major contiguous, cast fp32r.
            # Pair copies (two a-slices per op): src inner dim is an 8-byte
            # contiguous pair, dst writes the two a-major runs.
            xg = gpool.tile([P, NM * N_FEAT], DT_R)
            xg3 = xg[:].rearrange("p (a m) -> p m a", m=NM)
            for a0 in range(0, N_FEAT, 2):
                dst = xg3[:, :, a0:a0 + 2]
                src = x3[:, :, a0:a0 + 2]
                if a0 < 8:
                    nc.vector.tensor_copy(dst, src)
                else:
                    nc.scalar.copy(dst, src)

            o_sb = opool.tile([P, NM * 2], DT)
            o3 = o_sb[:].rearrange("p (m k) -> p m k", k=2)
            for k in range(2):
                ps = psum.tile([P, NM], mybir.dt.float32,
                               name=f"ps_{i}_{k}", tag=f"ps{k}")
                for a in range(N_FEAT):
                    j = k * 10 + a
                    nc.tensor.matmul(
                        ps[:],
                        w_sb[:, j * P:(j + 1) * P],
                        xg[:, a * NM:(a + 1) * NM],
                        start=(a == 0), stop=(a == N_FEAT - 1),
                    )
                nc.scalar.copy(o3[:, :, k], ps[:])

            # SWDGE (gpsimd) for the store: keeps the Sync queue free to
            # prefetch x tiles, and the gpsimd sequencer is otherwise idle
            # (issuing stores from ACT's HWDGE ring serializes behind its
            # ACTIVATE ops and measures ~7us slower end-to-end).
            nc.gpsimd.dma_start(o_t, o_sb[:])

    nc.compile()
    _CACHE["nc"] = nc
    return nc


def _host_m(psi_real, psi_imag, A_real, A_imag):
    """M_r in float64: the coefficient matrix multiplying x_real."""
    pr = psi_real.astype(np.float64)
    pi = psi_imag.astype(np.float64)
    Ar = A_real.astype(np.float64)
    Ai = A_imag.astype(np.float64)

    def mat(p1, A, p2):
        return np.einsum("i,kija,j->ka", p1, A, p2)

    M = (mat(pr, Ar, pr) - mat(pi, Ai, pr)
         - mat(pr, Ar, pi) + mat(pi, Ai, pi))
    return M.astype(np.float32)   # [2, 10]


def kernel(x, psi_real, psi_imag, A_real, A_imag, _trace=False):
    M = _host_m(psi_real, psi_imag, A_real, A_imag)

    x = np.ascontiguousarray(x, dtype=np.float32)
    x_pad = np.zeros((T_PAD, N_FEAT), dtype=np.float32)
    x_pad[:T] = x

    m_rep = np.tile(M.reshape(1, 20), (P, 1)).astype(np.float32)
    idm = np.eye(P, dtype=np.float32)

    nc = _build()
    in_maps = [
        {"x": x_pad[c * R:(c + 1) * R], "m": m_rep, "idm": idm}
        for c in range(N_CORES)
    ]
    res = run_bass_kernel_spmd(nc, in_maps, core_ids=list(range(N_CORES)),
                               trace=_trace)
    out = np.concatenate([res.results[c]["out"] for c in range(N_CORES)], axis=0)
    if _trace:
        kernel.last_results = res
    return out[:T]
